# revision 14
# baseline (speedup 1.0000x reference)
"""Causal self-attention on 8 Trainium2 NeuronCores.

Problem: B=4, T=2048, C=1024, H=16 (head_dim=64), fp32 I/O.

Sharding: tensor-parallel over heads. Core c owns heads (2c, 2c+1) =
channel slice [128c, 128c+128). Each core:
  - computes Q^T, K^T (layout [128 chan, T], head pair stacked 64+64 on
    partitions) and V ([T, 128+2 ones-cols]) for its head pair, all batches
  - scores S^T[tk, tq] = K_tile @ Q^T per (q-group of 512, key tile), both
    heads concurrently via PE row-tiling (contraction=64 each)
  - P^T = exp(S^T/8) on ACT (no max subtraction needed: |scores| < ~8),
    causal mask applied multiplicatively on the diagonal 128x128 block
  - o^T[65, tq] += V_aug^T @ P^T accumulated in PSUM over key tiles; the
    appended ones-column of V_aug yields softmax denominators in row 64
  - normalize, out-proj partial = y^T.T @ Wp[slice] -> fp32 partial output
Host sums the 8 partials and adds bp (the "all-reduce after proj" done at
unshard time). No device collectives.
"""

import numpy as np
import ml_dtypes

import concourse.bass as bass
import concourse.tile as tile
from concourse import mybir
from concourse.bass import ts, ds
from concourse.bass_utils import run_bass_kernel_spmd
from concourse.masks import make_upper_triangular, make_identity

B, T, C, H = 4, 2048, 1024, 16
HD = C // H          # 64
NCORES = 8
CPC = C // NCORES    # 128 channels per core (2 heads)
P = 128
KT = C // P          # 8 contraction tiles for projections
NT = T // P          # 16 token tiles per batch
NG = T // 512        # 4 query groups of 512 tokens per batch
F32 = mybir.dt.float32
BF16 = mybir.dt.bfloat16

_CACHED = {}


def _build_module(use_qk_bias: bool, use_v_bias: bool):
    from concourse import bacc
    nc = bacc.Bacc("TRN2", target_bir_lowering=False, debug=False,
                   num_devices=NCORES)

    xT = nc.dram_tensor("xT", (B, C, T), BF16, kind="ExternalInput").ap()
    wq = nc.dram_tensor("wq", (C, CPC), BF16, kind="ExternalInput").ap()
    wk = nc.dram_tensor("wk", (C, CPC), BF16, kind="ExternalInput").ap()
    wv = nc.dram_tensor("wv", (C, CPC), BF16, kind="ExternalInput").ap()
    wp = nc.dram_tensor("wp", (CPC, C), BF16, kind="ExternalInput").ap()
    bq = nc.dram_tensor("bq", (CPC, 1), F32, kind="ExternalInput").ap()
    bk = nc.dram_tensor("bk", (CPC, 1), F32, kind="ExternalInput").ap()
    bv = nc.dram_tensor("bv", (1, CPC), F32, kind="ExternalInput").ap()
    out = nc.dram_tensor("out", (B, T, C), F32, kind="ExternalOutput").ap()

    with tile.TileContext(nc) as tc:
        _emit(tc, xT, wq, wk, wv, wp, bq, bk, bv, out,
              use_qk_bias, use_v_bias)
    nc.compile()
    return nc


def _emit(tc, xT, wq, wk, wv, wp, bq, bk, bv, out, use_qk_bias, use_v_bias):
    from contextlib import ExitStack
    nc = tc.nc
    Exp = mybir.ActivationFunctionType.Exp
    Copy = mybir.ActivationFunctionType.Copy
    Ident = mybir.ActivationFunctionType.Identity
    scale = 1.0 / np.sqrt(HD)

    ctx = ExitStack()
    consts = ctx.enter_context(tc.tile_pool(name="consts", bufs=1))
    pool_x = ctx.enter_context(tc.tile_pool(name="x", bufs=2))
    pool_kt = ctx.enter_context(tc.tile_pool(name="kt", bufs=2))
    pool_qt = ctx.enter_context(tc.tile_pool(name="qt", bufs=2))
    pool_v = ctx.enter_context(tc.tile_pool(name="v", bufs=2))
    pool_y = ctx.enter_context(tc.tile_pool(name="y", bufs=2))
    pool_p = ctx.enter_context(tc.tile_pool(name="p", bufs=4))
    pool_o = ctx.enter_context(tc.tile_pool(name="osb", bufs=3))
    pool_sm = ctx.enter_context(tc.tile_pool(name="small", bufs=4))
    ps_big = ctx.enter_context(tc.tile_pool(name="ps_big", bufs=2, space="PSUM"))
    ps_s = ctx.enter_context(tc.tile_pool(name="ps_s", bufs=2, space="PSUM"))
    ps_o = ctx.enter_context(tc.tile_pool(name="ps_o", bufs=1, space="PSUM"))

    # ---- constants ----
    # Multiplicative causal mask in [tk, tq] coords: 1 where tq >= tk.
    mask_ut = consts.tile([P, P], BF16)
    make_upper_triangular(nc, mask_ut[:], val=1.0, diag=True)

    identity = consts.tile([P, P], F32)
    make_identity(nc, identity[:])

    wq_sb = consts.tile([P, KT, CPC], BF16)
    nc.sync.dma_start(wq_sb[:], wq.rearrange("(kt p) c -> p kt c", p=P))
    wk_sb = consts.tile([P, KT, CPC], BF16)
    nc.sync.dma_start(wk_sb[:], wk.rearrange("(kt p) c -> p kt c", p=P))
    wv_sb = consts.tile([P, KT, CPC], BF16)
    nc.sync.dma_start(wv_sb[:], wv.rearrange("(kt p) c -> p kt c", p=P))
    wp_sb = consts.tile([P, C], BF16)
    nc.sync.dma_start(wp_sb[:], wp)

    if use_qk_bias:
        bq_sb = consts.tile([P, 1], F32)
        nc.sync.dma_start(bq_sb[:], bq)
        bk_sb = consts.tile([P, 1], F32)
        nc.sync.dma_start(bk_sb[:], bk)
    if use_v_bias:
        bv_row = consts.tile([1, CPC], F32)
        nc.sync.dma_start(bv_row[:], bv)
        bv_bc = consts.tile([P, CPC], F32)
        nc.gpsimd.partition_broadcast(bv_bc[:], bv_row[:])

    for b in range(B):
        # ---- load x^T for this batch ----
        x_sb = pool_x.tile([P, KT, T], BF16)
        nc.sync.dma_start(x_sb[:], xT[b].rearrange("(kt p) t -> p kt t", p=P))

        # ---- K^T, Q^T projections: [128 chan, T] ----
        kt_sb = pool_kt.tile([P, T], BF16)
        qt_sb = pool_qt.tile([P, T], BF16)
        for (w_sb, o_sb, b_sb) in (
            (wk_sb, kt_sb, bk_sb if use_qk_bias else None),
            (wq_sb, qt_sb, bq_sb if use_qk_bias else None),
        ):
            for nchk in range(T // 512):
                ps = ps_big.tile([P, 512], F32, tag="big")
                for k in range(KT):
                    nc.tensor.matmul(ps[:], w_sb[:, k], x_sb[:, k, ts(nchk, 512)],
                                     start=(k == 0), stop=(k == KT - 1))
                if b_sb is not None:
                    nc.scalar.activation(o_sb[:, ts(nchk, 512)], ps[:], Ident,
                                         bias=b_sb[:])
                else:
                    nc.scalar.activation(o_sb[:, ts(nchk, 512)], ps[:], Copy)

        # ---- V projection: [t-tile, head dims + ones col] ----
        # layout: [128, NT, 130]; head A dims 0:64, ones at 64,
        #         head B dims 65:129, ones at 129.
        v_sb = pool_v.tile([P, NT, 2 * (HD + 1)], BF16)
        nc.vector.memset(v_sb[:, :, HD], 1.0)
        nc.vector.memset(v_sb[:, :, 2 * HD + 1], 1.0)
        for tt in range(NT):
            ps = ps_big.tile([P, 512], F32, tag="big")
            for k in range(KT):
                nc.tensor.matmul(ps[:, :CPC], x_sb[:, k, ts(tt, P)], wv_sb[:, k],
                                 start=(k == 0), stop=(k == KT - 1))
            if use_v_bias:
                nc.vector.tensor_add(ps[:, :CPC], ps[:, :CPC], bv_bc[:])
            nc.vector.tensor_copy(v_sb[:, tt, 0:HD], ps[:, 0:HD])
            nc.vector.tensor_copy(v_sb[:, tt, HD + 1:2 * HD + 1], ps[:, HD:CPC])

        # ---- attention per query group of 512 tokens ----
        # y layout: [tq-partition, token-tile, head-pair dims] fp32
        y_sb = pool_y.tile([P, NT, P], F32)
        for g in range(NG):
            o_ps = [ps_o.tile([HD + 1, 512], F32, tag=f"o{h}", name=f"o_ps{h}")
                    for h in (0, 1)]
            jmax = 4 * g + 3
            for j in range(jmax + 1):
                qs = max(512 * g, P * j)          # first visible query token
                nvis = 512 * (g + 1) - qs
                ql = qs - 512 * g
                diag = P * j >= 512 * g
                for h in (0, 1):
                    hp = 64 * h
                    s_ps = ps_s.tile([P, 512], F32, tag=f"s{h}")
                    nc.tensor.matmul(
                        s_ps[:, :nvis],
                        kt_sb[hp:hp + 64, ts(j, P)],
                        qt_sb[hp:hp + 64, ds(qs, nvis)],
                        start=True, stop=True,
                        tile_position=(hp, 0),
                    )
                    p_sb = pool_p.tile([P, 512], BF16, tag=f"p{h}")
                    nc.scalar.activation(p_sb[:, :nvis], s_ps[:, :nvis], Exp,
                                         scale=scale)
                    if diag:
                        nc.vector.tensor_mul(p_sb[:, 0:P], p_sb[:, 0:P],
                                             mask_ut[:])
                    nc.tensor.matmul(
                        o_ps[h][:, ds(ql, nvis)],
                        v_sb[:, j, (HD + 1) * h:(HD + 1) * (h + 1)],
                        p_sb[:, :nvis],
                        start=(j == 0), stop=(j == jmax),
                    )
            # normalize: transpose o^T back to [tq, dh+1]; row 64 holds the
            # softmax denominators, which become per-partition scalars.
            for h in (0, 1):
                osb = pool_sm.tile([HD + 1, 512], F32, tag="osb", name="osb")
                nc.vector.tensor_copy(osb[:], o_ps[h][:])
                for q in range(4):
                    tt = 4 * g + q
                    ot_ps = ps_s.tile([P, HD + 1], F32, tag=f"s{h}",
                                      name="ot_ps")
                    nc.tensor.transpose(ot_ps[:], osb[:, ts(q, P)],
                                        identity[0:HD + 1, 0:HD + 1])
                    rec = pool_sm.tile([P, 1], F32, tag="rec", name="rec")
                    nc.vector.reciprocal(rec[:], ot_ps[:, HD:HD + 1])
                    nc.vector.tensor_scalar_mul(
                        y_sb[:, tt, 64 * h:64 * h + HD],
                        ot_ps[:, 0:HD], rec[:])

        # ---- output projection partial: out[b] += y @ Wp[slice] ----
        for tt in range(NT):
            yt_ps = ps_s.tile([P, P], F32, tag="s0", name="yt_ps")
            nc.tensor.transpose(yt_ps[:], y_sb[:, tt, :], identity[:])
            ytsb = pool_p.tile([P, P], BF16, tag="p0", name="ytsb")
            nc.vector.tensor_copy(ytsb[:], yt_ps[:])
            for nchk in range(C // 512):
                ps = ps_big.tile([P, 512], F32, tag="big")
                nc.tensor.matmul(ps[:], ytsb[:],
                                 wp_sb[:, ts(nchk, 512)],
                                 start=True, stop=True)
                o_sb = pool_o.tile([P, 512], F32)
                if tt % 2 == 0:
                    nc.scalar.activation(o_sb[:], ps[:], Copy)
                else:
                    nc.vector.tensor_copy(o_sb[:], ps[:])
                nc.sync.dma_start(out[b, ts(tt, P), ts(nchk, 512)], o_sb[:])

    ctx.close()


def _get_module(use_qk_bias, use_v_bias):
    key = (use_qk_bias, use_v_bias)
    if key not in _CACHED:
        _CACHED[key] = _build_module(*key)
    return _CACHED[key]


def kernel(x, Wq, bq, Wk, bk, Wv, bv, Wp, bp):
    bf16 = ml_dtypes.bfloat16
    x = np.asarray(x, np.float32)
    Wq = np.asarray(Wq, np.float32)
    Wk = np.asarray(Wk, np.float32)
    Wv = np.asarray(Wv, np.float32)
    Wp = np.asarray(Wp, np.float32)
    bq = np.asarray(bq, np.float32)
    bk = np.asarray(bk, np.float32)
    bv = np.asarray(bv, np.float32)
    bp = np.asarray(bp, np.float32)

    use_qk_bias = bool(np.any(bq) or np.any(bk))
    use_v_bias = bool(np.any(bv))
    nc = _get_module(use_qk_bias, use_v_bias)

    xT = np.ascontiguousarray(x.transpose(0, 2, 1)).astype(bf16)
    in_maps = []
    for c in range(NCORES):
        sl = slice(CPC * c, CPC * (c + 1))
        in_maps.append({
            "xT": xT,
            "wq": np.ascontiguousarray(Wq[:, sl]).astype(bf16),
            "wk": np.ascontiguousarray(Wk[:, sl]).astype(bf16),
            "wv": np.ascontiguousarray(Wv[:, sl]).astype(bf16),
            "wp": np.ascontiguousarray(Wp[sl, :]).astype(bf16),
            "bq": np.ascontiguousarray(bq[sl]).reshape(CPC, 1),
            "bk": np.ascontiguousarray(bk[sl]).reshape(CPC, 1),
            "bv": np.ascontiguousarray(bv[sl]).reshape(1, CPC),
        })

    res = run_bass_kernel_spmd(nc, in_maps, core_ids=list(range(NCORES)))
    global LAST_RESULT
    LAST_RESULT = res
    y = res.results[0]["out"].astype(np.float64)
    for c in range(1, NCORES):
        y += res.results[c]["out"]
    y += bp
    return y.astype(np.float32)


# revision 16
# speedup vs baseline: 1.3084x; 1.3084x over previous
"""Causal self-attention on 8 Trainium2 NeuronCores.

Problem: B=4, T=2048, C=1024, H=16 (head_dim=64), fp32 I/O.

Sharding: tensor-parallel over heads. Core c owns heads (2c, 2c+1) =
channel slice [128c, 128c+128). Each core:
  - computes Q^T, K^T (layout [128 chan, T], head pair stacked 64+64 on
    partitions) and V ([T, 128+2 ones-cols]) for its head pair, all batches
  - scores S^T[tk, tq] = K_tile @ Q^T per (q-group of 512, key tile), both
    heads concurrently via PE row-tiling (contraction=64 each) into a
    2-bank PSUM tensor
  - P^T = exp(S^T/8) on ACT, one instruction per key tile covering both
    heads (no max subtraction needed: |scores| < ~8); causal mask applied
    multiplicatively on the diagonal 128x128 block
  - o^T[65, tq] += V_aug^T @ P^T accumulated in PSUM over key tiles; the
    appended ones-column of V_aug yields softmax denominators in row 64
  - normalize by transposing o^T to [tq, dh+1] (PE transpose), reciprocal
    of the per-partition denominator column, per-partition scalar multiply
  - out-proj partial = y^T.T @ Wp[slice] -> fp32 partial output

The PE instruction stream is software-pipelined: batch b's attention
steps are interleaved with batch b+1's Q/K/V projection chains and batch
b's out-projection chunks ("filler" units), keeping TensorE dense so the
HAM clock gate stays at full rate.

Host sums the 8 partial outputs and adds bp (the "all-reduce after proj"
done at unshard time). No device collectives.
"""

from collections import deque
from contextlib import ExitStack

import numpy as np
import ml_dtypes

import concourse.bass as bass
import concourse.tile as tile
from concourse import mybir
from concourse.bass import ts, ds
from concourse.bass_utils import run_bass_kernel_spmd
from concourse.masks import make_upper_triangular, make_identity

B, T, C, H = 4, 2048, 1024, 16
HD = C // H          # 64
NCORES = 8
CPC = C // NCORES    # 128 channels per core (2 heads)
P = 128
KT = C // P          # 8 contraction tiles for projections
NT = T // P          # 16 token tiles per batch
NG = T // 512        # 4 query groups of 512 tokens per batch
F32 = mybir.dt.float32
BF16 = mybir.dt.bfloat16

_CACHED = {}
LAST_RESULT = None


def _build_module(use_qk_bias: bool, use_v_bias: bool):
    from concourse import bacc
    nc = bacc.Bacc("TRN2", target_bir_lowering=False, debug=False,
                   num_devices=NCORES)

    xT = nc.dram_tensor("xT", (B, C, T), BF16, kind="ExternalInput").ap()
    wq = nc.dram_tensor("wq", (C, CPC), BF16, kind="ExternalInput").ap()
    wk = nc.dram_tensor("wk", (C, CPC), BF16, kind="ExternalInput").ap()
    wv = nc.dram_tensor("wv", (C, CPC), BF16, kind="ExternalInput").ap()
    wp = nc.dram_tensor("wp", (CPC, C), BF16, kind="ExternalInput").ap()
    bq = nc.dram_tensor("bq", (CPC, 1), F32, kind="ExternalInput").ap()
    bk = nc.dram_tensor("bk", (CPC, 1), F32, kind="ExternalInput").ap()
    bv = nc.dram_tensor("bv", (1, CPC), F32, kind="ExternalInput").ap()
    out = nc.dram_tensor("out", (B, T, C), F32, kind="ExternalOutput").ap()

    with tile.TileContext(nc) as tc:
        _emit(tc, xT, wq, wk, wv, wp, bq, bk, bv, out,
              use_qk_bias, use_v_bias)
    nc.compile()
    return nc


def _emit(tc, xT, wq, wk, wv, wp, bq, bk, bv, out, use_qk_bias, use_v_bias):
    nc = tc.nc
    Exp = mybir.ActivationFunctionType.Exp
    Copy = mybir.ActivationFunctionType.Copy
    Ident = mybir.ActivationFunctionType.Identity
    scale = 1.0 / np.sqrt(HD)

    ctx = ExitStack()
    consts = ctx.enter_context(tc.tile_pool(name="consts", bufs=1))
    pool_x = ctx.enter_context(tc.tile_pool(name="x", bufs=2))
    pool_kt = ctx.enter_context(tc.tile_pool(name="kt", bufs=2))
    pool_qt = ctx.enter_context(tc.tile_pool(name="qt", bufs=2))
    pool_v = ctx.enter_context(tc.tile_pool(name="v", bufs=2))
    pool_y = ctx.enter_context(tc.tile_pool(name="y", bufs=2))
    pool_p = ctx.enter_context(tc.tile_pool(name="p", bufs=4))
    pool_o = ctx.enter_context(tc.tile_pool(name="osb", bufs=3))
    pool_sm = ctx.enter_context(tc.tile_pool(name="small", bufs=4))
    ps_big = ctx.enter_context(tc.tile_pool(name="ps_big", bufs=2, space="PSUM"))
    ps_s = ctx.enter_context(tc.tile_pool(name="ps_s", bufs=2, space="PSUM"))
    ps_o = ctx.enter_context(tc.tile_pool(name="ps_o", bufs=1, space="PSUM"))

    # ---- constants ----
    # Multiplicative causal mask in [tk, tq] coords: 1 where tq >= tk.
    mask_ut = consts.tile([P, P], BF16)
    make_upper_triangular(nc, mask_ut[:], val=1.0, diag=True)

    identity = consts.tile([P, P], F32)
    make_identity(nc, identity[:])

    wq_sb = consts.tile([P, KT, CPC], BF16)
    nc.sync.dma_start(wq_sb[:], wq.rearrange("(kt p) c -> p kt c", p=P))
    wk_sb = consts.tile([P, KT, CPC], BF16)
    nc.sync.dma_start(wk_sb[:], wk.rearrange("(kt p) c -> p kt c", p=P))
    wv_sb = consts.tile([P, KT, CPC], BF16)
    nc.sync.dma_start(wv_sb[:], wv.rearrange("(kt p) c -> p kt c", p=P))
    wp_sb = consts.tile([P, C], BF16)
    nc.sync.dma_start(wp_sb[:], wp)

    if use_qk_bias:
        bq_sb = consts.tile([P, 1], F32)
        nc.sync.dma_start(bq_sb[:], bq)
        bk_sb = consts.tile([P, 1], F32)
        nc.sync.dma_start(bk_sb[:], bk)
    if use_v_bias:
        bv_row = consts.tile([1, CPC], F32)
        nc.sync.dma_start(bv_row[:], bv)
        ones_col = consts.tile([1, P], F32)
        nc.vector.memset(ones_col[:], 1.0)
        ps_bv = ps_big.tile([P, 512], F32, tag="big", name="ps_bv")
        nc.tensor.matmul(ps_bv[:, :CPC], ones_col[:], bv_row[:],
                         start=True, stop=True)
        bv_bc = consts.tile([P, CPC], F32)
        nc.vector.tensor_copy(bv_bc[:], ps_bv[:, :CPC])

    # per-batch resident tiles, allocated lazily (2 batches live at a time)
    state = {}

    def ensure_batch(b):
        if b in state:
            return state[b]
        st = {}
        st["x"] = pool_x.tile([P, KT, T], BF16, tag="x", name=f"x{b}")
        for k in range(KT):
            nc.sync.dma_start(
                st["x"][:, k],
                xT[b].rearrange("(kt p) t -> p kt t", p=P)[:, k])
        st["kt"] = pool_kt.tile([P, T], BF16, tag="kt", name=f"kt{b}")
        st["qt"] = pool_qt.tile([P, T], BF16, tag="qt", name=f"qt{b}")
        st["v"] = pool_v.tile([P, NT, 2 * (HD + 1)], BF16, tag="v",
                              name=f"v{b}")
        nc.vector.memset(st["v"][:, :, HD], 1.0)
        nc.vector.memset(st["v"][:, :, 2 * HD + 1], 1.0)
        st["y"] = pool_y.tile([P, NT, P], F32, tag="y", name=f"y{b}")
        state[b] = st
        return st

    def kq_unit(b, which, nchk):
        """One 512-column chunk of the K^T or Q^T projection of batch b."""
        def run():
            st = state[b]
            w_sb = wk_sb if which == "k" else wq_sb
            o_sb = st["kt"] if which == "k" else st["qt"]
            b_sb = None
            if use_qk_bias:
                b_sb = bk_sb if which == "k" else bq_sb
            ps = ps_big.tile([P, 512], F32, tag="big", name=f"ps_{which}")
            for k in range(KT):
                nc.tensor.matmul(ps[:], w_sb[:, k],
                                 st["x"][:, k, ts(nchk, 512)],
                                 start=(k == 0), stop=(k == KT - 1))
            if b_sb is not None:
                nc.scalar.activation(o_sb[:, ts(nchk, 512)], ps[:], Ident,
                                     bias=b_sb[:])
            else:
                nc.scalar.activation(o_sb[:, ts(nchk, 512)], ps[:], Copy)
        return run

    def v_unit(b, tt):
        """One 128-token tile of the V projection of batch b."""
        def run():
            st = state[b]
            ps = ps_big.tile([P, 512], F32, tag="big", name="ps_v")
            for k in range(KT):
                nc.tensor.matmul(ps[:, :CPC], st["x"][:, k, ts(tt, P)],
                                 wv_sb[:, k],
                                 start=(k == 0), stop=(k == KT - 1))
            if use_v_bias:
                nc.vector.tensor_add(ps[:, :CPC], ps[:, :CPC], bv_bc[:])
            # one strided copy: psum [128,(2,64)] -> v cols {0:64, 65:129}
            dst = st["v"][:, tt, :].rearrange("p (h c) -> p h c", c=HD + 1)
            nc.vector.tensor_copy(
                dst[:, :, 0:HD],
                ps[:, :CPC].rearrange("p (h c) -> p h c", c=HD))
        return run

    def outproj_unit(b, tt):
        """Out-projection of one 128-token tile of batch b."""
        def run():
            st = state[b]
            yt_ps = ps_s.tile([P, P], F32, tag="s", name="yt_ps")
            nc.tensor.transpose(yt_ps[:], st["y"][:, tt, :], identity[:])
            ytsb = pool_p.tile([P, P], BF16, tag="p", name="ytsb")
            nc.vector.tensor_copy(ytsb[:], yt_ps[:])
            for nchk in range(C // 512):
                ps = ps_big.tile([P, 512], F32, tag="big", name="ps_op")
                nc.tensor.matmul(ps[:], ytsb[:], wp_sb[:, ts(nchk, 512)],
                                 start=True, stop=True)
                o_sb = pool_o.tile([P, 512], F32, tag="osb2", name="o_sb")
                if (tt + nchk) % 2 == 0:
                    nc.scalar.activation(o_sb[:], ps[:], Copy)
                else:
                    nc.vector.tensor_copy(o_sb[:], ps[:])
                nc.sync.dma_start(out[b, ts(tt, P), ts(nchk, 512)], o_sb[:])
        return run

    def proj_units(b):
        units = []
        for which in ("k", "q"):
            for nchk in range(T // 512):
                units.append(kq_unit(b, which, nchk))
        for tt in range(NT):
            units.append(v_unit(b, tt))
        return units

    def attn_batch(b, fillers):
        """Attention for batch b; pops one filler after each key-tile step
        and pushes out-projection units as query groups complete."""
        st = state[b]
        kt_sb, qt_sb, v_sb, y_sb = st["kt"], st["qt"], st["v"], st["y"]
        for g in range(NG):
            o_ps = [ps_o.tile([HD + 1, 512], F32, tag=f"o{h}",
                              name=f"o_ps{h}") for h in (0, 1)]
            jmax = 4 * g + 3
            for j in range(jmax + 1):
                qs = max(512 * g, P * j)          # first visible query token
                nvis = 512 * (g + 1) - qs
                ql = qs - 512 * g
                diag = P * j >= 512 * g
                s_ps = ps_s.tile([P, 2, 512], F32, tag="s", name="s_ps")
                for h in (0, 1):
                    hp = 64 * h
                    nc.tensor.matmul(
                        s_ps[:, h, :nvis],
                        kt_sb[hp:hp + 64, ts(j, P)],
                        qt_sb[hp:hp + 64, ds(qs, nvis)],
                        start=True, stop=True,
                        tile_position=(hp, 0),
                    )
                p_sb = pool_p.tile([P, 2, 512], BF16, tag="p", name="p_sb")
                nc.scalar.activation(p_sb[:, :, :nvis], s_ps[:, :, :nvis],
                                     Exp, scale=scale)
                if diag:
                    nc.vector.tensor_mul(
                        p_sb[:, :, 0:P], p_sb[:, :, 0:P],
                        mask_ut[:, None, :].to_broadcast([P, 2, P]))
                for h in (0, 1):
                    nc.tensor.matmul(
                        o_ps[h][:, ds(ql, nvis)],
                        v_sb[:, j, (HD + 1) * h:(HD + 1) * (h + 1)],
                        p_sb[:, h, :nvis],
                        start=(j == 0), stop=(j == jmax),
                    )
                if fillers:
                    fillers.popleft()()
            # normalize: transpose o^T to [tq, dh+1]; row 64 holds the
            # softmax denominators -> per-partition scalars after transpose.
            for h in (0, 1):
                osb = pool_sm.tile([HD + 1, 512], F32, tag="osb", name="osb")
                nc.vector.tensor_copy(osb[:], o_ps[h][:])
                for q in range(4):
                    tt = 4 * g + q
                    ot_ps = ps_s.tile([P, HD + 1], F32, tag="s",
                                      name="ot_ps")
                    nc.tensor.transpose(ot_ps[:], osb[:, ts(q, P)],
                                        identity[0:HD + 1, 0:HD + 1])
                    rec = pool_sm.tile([P, 1], F32, tag="rec", name="rec")
                    nc.vector.reciprocal(rec[:], ot_ps[:, HD:HD + 1])
                    nc.vector.tensor_scalar_mul(
                        y_sb[:, tt, 64 * h:64 * h + HD],
                        ot_ps[:, 0:HD], rec[:])
            for tt in range(4 * g, 4 * g + 4):
                fillers.append(outproj_unit(b, tt))

    # ---- schedule ----
    ensure_batch(0)
    for u in proj_units(0):          # prologue: dense projections, warm PE
        u()
    for b in range(B):
        fillers = deque()
        if b + 1 < B:
            ensure_batch(b + 1)
            fillers.extend(proj_units(b + 1))
        attn_batch(b, fillers)
        while fillers:
            fillers.popleft()()

    ctx.close()


def _get_module(use_qk_bias, use_v_bias):
    key = (use_qk_bias, use_v_bias)
    if key not in _CACHED:
        _CACHED[key] = _build_module(*key)
    return _CACHED[key]


def kernel(x, Wq, bq, Wk, bk, Wv, bv, Wp, bp):
    bf16 = ml_dtypes.bfloat16
    x = np.asarray(x, np.float32)
    Wq = np.asarray(Wq, np.float32)
    Wk = np.asarray(Wk, np.float32)
    Wv = np.asarray(Wv, np.float32)
    Wp = np.asarray(Wp, np.float32)
    bq = np.asarray(bq, np.float32)
    bk = np.asarray(bk, np.float32)
    bv = np.asarray(bv, np.float32)
    bp = np.asarray(bp, np.float32)

    use_qk_bias = bool(np.any(bq) or np.any(bk))
    use_v_bias = bool(np.any(bv))
    nc = _get_module(use_qk_bias, use_v_bias)

    xT = np.ascontiguousarray(x.transpose(0, 2, 1)).astype(bf16)
    in_maps = []
    for c in range(NCORES):
        sl = slice(CPC * c, CPC * (c + 1))
        in_maps.append({
            "xT": xT,
            "wq": np.ascontiguousarray(Wq[:, sl]).astype(bf16),
            "wk": np.ascontiguousarray(Wk[:, sl]).astype(bf16),
            "wv": np.ascontiguousarray(Wv[:, sl]).astype(bf16),
            "wp": np.ascontiguousarray(Wp[sl, :]).astype(bf16),
            "bq": np.ascontiguousarray(bq[sl]).reshape(CPC, 1),
            "bk": np.ascontiguousarray(bk[sl]).reshape(CPC, 1),
            "bv": np.ascontiguousarray(bv[sl]).reshape(1, CPC),
        })

    res = run_bass_kernel_spmd(nc, in_maps, core_ids=list(range(NCORES)))
    global LAST_RESULT
    LAST_RESULT = res
    y = res.results[0]["out"].astype(np.float64)
    for c in range(1, NCORES):
        y += res.results[c]["out"]
    y += bp
    return y.astype(np.float32)


# revision 21
# speedup vs baseline: 1.3687x; 1.0461x over previous
"""Causal self-attention on 8 Trainium2 NeuronCores.

Problem: B=4, T=2048, C=1024, H=16 (head_dim=64), fp32 I/O.

Sharding: tensor-parallel over heads. Core c owns heads (2c, 2c+1) =
channel slice [128c, 128c+128). Each core:
  - computes Q^T, K^T (layout [128 chan, T], head pair stacked 64+64 on
    partitions) and V ([T, 128+2 ones-cols]) for its head pair, all batches
  - scores S^T[tk, tq] = K_tile @ Q^T per (q-group of 512, key tile), both
    heads concurrently via PE row-tiling (contraction=64 each) into a
    2-bank PSUM tensor
  - P^T = exp(S^T/8) on ACT, one instruction per key tile covering both
    heads (no max subtraction needed: |scores| < ~8); causal mask applied
    multiplicatively on the diagonal 128x128 block
  - o^T[65, tq] += V_aug^T @ P^T accumulated in PSUM over key tiles; the
    appended ones-column of V_aug yields softmax denominators in row 64
  - normalize by transposing o^T to [tq, dh+1] (PE transpose), reciprocal
    of the per-partition denominator column, per-partition scalar multiply
  - out-proj partial = y^T.T @ Wp[slice] -> fp32 partial output

The PE instruction stream is software-pipelined: batch b's attention
steps are interleaved with batch b+1's Q/K/V projection chains and batch
b's out-projection chunks ("filler" units), keeping TensorE dense so the
HAM clock gate stays at full rate.

Host sums the 8 partial outputs and adds bp (the "all-reduce after proj"
done at unshard time). No device collectives.
"""

from collections import deque
from contextlib import ExitStack

import numpy as np
import ml_dtypes

import concourse.bass as bass
import concourse.tile as tile
from concourse import mybir
from concourse.bass import ts, ds
from concourse.bass_utils import run_bass_kernel_spmd
from concourse.masks import make_upper_triangular, make_identity

B, T, C, H = 4, 2048, 1024, 16
HD = C // H          # 64
NCORES = 8
CPC = C // NCORES    # 128 channels per core (2 heads)
P = 128
KT = C // P          # 8 contraction tiles for projections
NT = T // P          # 16 token tiles per batch
NG = T // 512        # 4 query groups of 512 tokens per batch
F32 = mybir.dt.float32
BF16 = mybir.dt.bfloat16

_CACHED = {}
LAST_RESULT = None


def _build_module(use_qk_bias: bool, use_v_bias: bool):
    from concourse import bacc
    nc = bacc.Bacc("TRN2", target_bir_lowering=False, debug=False,
                   num_devices=NCORES)

    xT = nc.dram_tensor("xT", (B, C, T), BF16, kind="ExternalInput").ap()
    wq = nc.dram_tensor("wq", (C, CPC), BF16, kind="ExternalInput").ap()
    wk = nc.dram_tensor("wk", (C, CPC), BF16, kind="ExternalInput").ap()
    wv = nc.dram_tensor("wv", (C, CPC), BF16, kind="ExternalInput").ap()
    wp = nc.dram_tensor("wp", (CPC, C), BF16, kind="ExternalInput").ap()
    bq = nc.dram_tensor("bq", (CPC, 1), F32, kind="ExternalInput").ap()
    bk = nc.dram_tensor("bk", (CPC, 1), F32, kind="ExternalInput").ap()
    bv = nc.dram_tensor("bv", (1, CPC), F32, kind="ExternalInput").ap()
    out = nc.dram_tensor("out", (B, T, C), F32, kind="ExternalOutput").ap()

    with tile.TileContext(nc) as tc:
        _emit(tc, xT, wq, wk, wv, wp, bq, bk, bv, out,
              use_qk_bias, use_v_bias)
    nc.compile()
    return nc


def _emit(tc, xT, wq, wk, wv, wp, bq, bk, bv, out, use_qk_bias, use_v_bias):
    nc = tc.nc
    Exp = mybir.ActivationFunctionType.Exp
    Copy = mybir.ActivationFunctionType.Copy
    Ident = mybir.ActivationFunctionType.Identity
    scale = 1.0 / np.sqrt(HD)

    ctx = ExitStack()
    consts = ctx.enter_context(tc.tile_pool(name="consts", bufs=1))
    pool_x = ctx.enter_context(tc.tile_pool(name="x", bufs=2))
    pool_kt = ctx.enter_context(tc.tile_pool(name="kt", bufs=2))
    pool_qt = ctx.enter_context(tc.tile_pool(name="qt", bufs=2))
    pool_v = ctx.enter_context(tc.tile_pool(name="v", bufs=2))
    pool_y = ctx.enter_context(tc.tile_pool(name="y", bufs=3))
    pool_p = ctx.enter_context(tc.tile_pool(name="p", bufs=4))
    pool_o = ctx.enter_context(tc.tile_pool(name="osb", bufs=3))
    pool_sm = ctx.enter_context(tc.tile_pool(name="small", bufs=4))
    ps_big = ctx.enter_context(tc.tile_pool(name="ps_big", bufs=2, space="PSUM"))
    ps_s = ctx.enter_context(tc.tile_pool(name="ps_s", bufs=2, space="PSUM"))
    ps_o = ctx.enter_context(tc.tile_pool(name="ps_o", bufs=1, space="PSUM"))

    # ---- constants ----
    # Multiplicative causal mask in [tk, tq] coords: 1 where tq >= tk.
    mask_ut = consts.tile([P, P], BF16)
    make_upper_triangular(nc, mask_ut[:], val=1.0, diag=True)

    identity = consts.tile([P, P], F32)
    make_identity(nc, identity[:])

    wq_sb = consts.tile([P, KT, CPC], BF16)
    nc.sync.dma_start(wq_sb[:], wq.rearrange("(kt p) c -> p kt c", p=P))
    wk_sb = consts.tile([P, KT, CPC], BF16)
    nc.sync.dma_start(wk_sb[:], wk.rearrange("(kt p) c -> p kt c", p=P))
    wv_sb = consts.tile([P, KT, CPC], BF16)
    nc.sync.dma_start(wv_sb[:], wv.rearrange("(kt p) c -> p kt c", p=P))
    wp_sb = consts.tile([P, C], BF16)
    nc.sync.dma_start(wp_sb[:], wp)

    if use_qk_bias:
        bq_sb = consts.tile([P, 1], F32)
        nc.sync.dma_start(bq_sb[:], bq)
        bk_sb = consts.tile([P, 1], F32)
        nc.sync.dma_start(bk_sb[:], bk)
    if use_v_bias:
        bv_row = consts.tile([1, CPC], F32)
        nc.sync.dma_start(bv_row[:], bv)
        ones_col = consts.tile([1, P], F32)
        nc.vector.memset(ones_col[:], 1.0)
        ps_bv = ps_big.tile([P, 512], F32, tag="big", name="ps_bv")
        nc.tensor.matmul(ps_bv[:, :CPC], ones_col[:], bv_row[:],
                         start=True, stop=True)
        bv_bc = consts.tile([P, CPC], F32)
        nc.vector.tensor_copy(bv_bc[:], ps_bv[:, :CPC])

    # per-batch resident tiles, allocated lazily (2 batches live at a time)
    state = {}

    def ensure_batch(b):
        if b in state:
            return state[b]
        st = {}
        st["x"] = pool_x.tile([P, KT, T], BF16, tag="x", name=f"x{b}")
        for k in range(KT):
            nc.sync.dma_start(
                st["x"][:, k],
                xT[b].rearrange("(kt p) t -> p kt t", p=P)[:, k])
        st["kt"] = pool_kt.tile([P, T], BF16, tag="kt", name=f"kt{b}")
        st["qt"] = pool_qt.tile([P, T], BF16, tag="qt", name=f"qt{b}")
        st["v"] = pool_v.tile([P, NT, 2 * (HD + 1)], BF16, tag="v",
                              name=f"v{b}")
        nc.vector.memset(st["v"][:, :, HD], 1.0)
        nc.vector.memset(st["v"][:, :, 2 * HD + 1], 1.0)
        st["y"] = pool_y.tile([P, NT, P], F32, tag="y", name=f"y{b}")
        state[b] = st
        return st

    def kq_unit(b, which, nchk):
        """One 512-column chunk of the K^T or Q^T projection of batch b."""
        def run():
            st = state[b]
            w_sb = wk_sb if which == "k" else wq_sb
            o_sb = st["kt"] if which == "k" else st["qt"]
            b_sb = None
            if use_qk_bias:
                b_sb = bk_sb if which == "k" else bq_sb
            ps = ps_big.tile([P, 512], F32, tag="big", name=f"ps_{which}")
            for k in range(KT):
                nc.tensor.matmul(ps[:], w_sb[:, k],
                                 st["x"][:, k, ts(nchk, 512)],
                                 start=(k == 0), stop=(k == KT - 1))
            if b_sb is not None:
                nc.scalar.activation(o_sb[:, ts(nchk, 512)], ps[:], Ident,
                                     bias=b_sb[:])
            elif which == "k":
                nc.scalar.activation(o_sb[:, ts(nchk, 512)], ps[:], Copy)
            else:
                nc.vector.tensor_copy(o_sb[:, ts(nchk, 512)], ps[:])
        return run

    def v_unit(b, tt):
        """One 128-token tile of the V projection of batch b."""
        def run():
            st = state[b]
            ps = ps_big.tile([P, 512], F32, tag="big", name="ps_v")
            for k in range(KT):
                nc.tensor.matmul(ps[:, :CPC], st["x"][:, k, ts(tt, P)],
                                 wv_sb[:, k],
                                 start=(k == 0), stop=(k == KT - 1))
            if use_v_bias:
                nc.vector.tensor_add(ps[:, :CPC], ps[:, :CPC], bv_bc[:])
            # one strided copy: psum [128,(2,64)] -> v cols {0:64, 65:129}
            dst = st["v"][:, tt, :].rearrange("p (h c) -> p h c", c=HD + 1)
            nc.vector.tensor_copy(
                dst[:, :, 0:HD],
                ps[:, :CPC].rearrange("p (h c) -> p h c", c=HD))
        return run

    def outproj_unit(b, tt):
        """Out-projection of one 128-token tile of batch b."""
        def run():
            st = state[b]
            yt_ps = ps_s.tile([P, P], F32, tag="s", name="yt_ps")
            nc.tensor.transpose(yt_ps[:], st["y"][:, tt, :], identity[:])
            ytsb = pool_p.tile([P, P], BF16, tag="p", name="ytsb")
            nc.vector.tensor_copy(ytsb[:], yt_ps[:])
            for nchk in range(C // 512):
                ps = ps_big.tile([P, 512], F32, tag="big", name="ps_op")
                nc.tensor.matmul(ps[:], ytsb[:], wp_sb[:, ts(nchk, 512)],
                                 start=True, stop=True)
                o_sb = pool_o.tile([P, 512], F32, tag="osb2", name="o_sb")
                if (tt + nchk) % 2 == 0:
                    nc.scalar.activation(o_sb[:], ps[:], Copy)
                else:
                    nc.vector.tensor_copy(o_sb[:], ps[:])
                nc.sync.dma_start(out[b, ts(tt, P), ts(nchk, 512)], o_sb[:])
        return run

    def proj_units(b):
        units = []
        for which in ("k", "q"):
            for nchk in range(T // 512):
                units.append(kq_unit(b, which, nchk))
        for tt in range(NT):
            units.append(v_unit(b, tt))
        return units

    def attn_batch(b, fillers_proj, fillers_free, allow_free):
        """Attention for batch b; pops one filler after each key-tile step
        and pushes out-projection units as query groups complete.
        Next-batch projection units (fillers_proj) are popped first; free
        units (out-projections) are popped only when allow_free, so they
        can be reserved as filler for the last batch."""
        st = state[b]
        kt_sb, qt_sb, v_sb, y_sb = st["kt"], st["qt"], st["v"], st["y"]
        for g in range(NG):
            o_ps = [ps_o.tile([HD + 1, 512], F32, tag=f"o{h}",
                              name=f"o_ps{h}") for h in (0, 1)]
            jmax = 4 * g + 3
            for j in range(jmax + 1):
                qs = max(512 * g, P * j)          # first visible query token
                nvis = 512 * (g + 1) - qs
                ql = qs - 512 * g
                diag = P * j >= 512 * g
                s_ps = ps_s.tile([P, 2, 512], F32, tag="s", name="s_ps")
                for h in (0, 1):
                    hp = 64 * h
                    nc.tensor.matmul(
                        s_ps[:, h, :nvis],
                        kt_sb[hp:hp + 64, ts(j, P)],
                        qt_sb[hp:hp + 64, ds(qs, nvis)],
                        start=True, stop=True,
                        tile_position=(hp, 0),
                    )
                p_sb = pool_p.tile([P, 2, 512], BF16, tag="p", name="p_sb")
                nc.scalar.activation(p_sb[:, :, :nvis], s_ps[:, :, :nvis],
                                     Exp, scale=scale)
                if diag:
                    nc.vector.tensor_mul(
                        p_sb[:, :, 0:P], p_sb[:, :, 0:P],
                        mask_ut[:, None, :].to_broadcast([P, 2, P]))
                for h in (0, 1):
                    nc.tensor.matmul(
                        o_ps[h][:, ds(ql, nvis)],
                        v_sb[:, j, (HD + 1) * h:(HD + 1) * (h + 1)],
                        p_sb[:, h, :nvis],
                        start=(j == 0), stop=(j == jmax),
                    )
                if fillers_proj:
                    fillers_proj.popleft()()
                elif allow_free and fillers_free:
                    fillers_free.popleft()()
            # normalize: transpose o^T to [tq, dh+1]; row 64 holds the
            # softmax denominators -> per-partition scalars after transpose.
            for h in (0, 1):
                osb = pool_sm.tile([HD + 1, 512], F32, tag="osb", name="osb")
                nc.vector.tensor_copy(osb[:], o_ps[h][:])
                for q in range(4):
                    tt = 4 * g + q
                    ot_ps = ps_s.tile([P, HD + 1], F32, tag="s",
                                      name="ot_ps")
                    nc.tensor.transpose(ot_ps[:], osb[:, ts(q, P)],
                                        identity[0:HD + 1, 0:HD + 1])
                    rec = pool_sm.tile([P, 1], F32, tag="rec", name="rec")
                    nc.vector.reciprocal(rec[:], ot_ps[:, HD:HD + 1])
                    nc.vector.tensor_scalar_mul(
                        y_sb[:, tt, 64 * h:64 * h + HD],
                        ot_ps[:, 0:HD], rec[:])
            for tt in range(4 * g, 4 * g + 4):
                fillers_free.append(outproj_unit(b, tt))

    # ---- schedule ----
    ensure_batch(0)
    for u in proj_units(0):          # prologue: dense projections, warm PE
        u()
    fillers_free = deque()
    for b in range(B):
        fillers_proj = deque()
        if b + 1 < B:
            ensure_batch(b + 1)
            fillers_proj.extend(proj_units(b + 1))
        # reserve out-projection units during the second-to-last batch so
        # the last batch's attention still has dense PE filler
        attn_batch(b, fillers_proj, fillers_free, allow_free=(b != B - 2))
        while fillers_proj:
            fillers_proj.popleft()()
    while fillers_free:
        fillers_free.popleft()()

    ctx.close()


def _get_module(use_qk_bias, use_v_bias):
    key = (use_qk_bias, use_v_bias)
    if key not in _CACHED:
        _CACHED[key] = _build_module(*key)
    return _CACHED[key]


def kernel(x, Wq, bq, Wk, bk, Wv, bv, Wp, bp):
    bf16 = ml_dtypes.bfloat16
    x = np.asarray(x, np.float32)
    Wq = np.asarray(Wq, np.float32)
    Wk = np.asarray(Wk, np.float32)
    Wv = np.asarray(Wv, np.float32)
    Wp = np.asarray(Wp, np.float32)
    bq = np.asarray(bq, np.float32)
    bk = np.asarray(bk, np.float32)
    bv = np.asarray(bv, np.float32)
    bp = np.asarray(bp, np.float32)

    use_qk_bias = bool(np.any(bq) or np.any(bk))
    use_v_bias = bool(np.any(bv))
    nc = _get_module(use_qk_bias, use_v_bias)

    xT = np.ascontiguousarray(x.transpose(0, 2, 1)).astype(bf16)
    in_maps = []
    for c in range(NCORES):
        sl = slice(CPC * c, CPC * (c + 1))
        in_maps.append({
            "xT": xT,
            "wq": np.ascontiguousarray(Wq[:, sl]).astype(bf16),
            "wk": np.ascontiguousarray(Wk[:, sl]).astype(bf16),
            "wv": np.ascontiguousarray(Wv[:, sl]).astype(bf16),
            "wp": np.ascontiguousarray(Wp[sl, :]).astype(bf16),
            "bq": np.ascontiguousarray(bq[sl]).reshape(CPC, 1),
            "bk": np.ascontiguousarray(bk[sl]).reshape(CPC, 1),
            "bv": np.ascontiguousarray(bv[sl]).reshape(1, CPC),
        })

    res = run_bass_kernel_spmd(nc, in_maps, core_ids=list(range(NCORES)))
    global LAST_RESULT
    LAST_RESULT = res
    y = res.results[0]["out"].astype(np.float64)
    for c in range(1, NCORES):
        y += res.results[c]["out"]
    y += bp
    return y.astype(np.float32)


# revision 23
# speedup vs baseline: 1.3791x; 1.0076x over previous
"""Causal self-attention on 8 Trainium2 NeuronCores.

Problem: B=4, T=2048, C=1024, H=16 (head_dim=64), fp32 I/O.

Sharding: tensor-parallel over heads. Core c owns heads (2c, 2c+1) =
channel slice [128c, 128c+128). Each core:
  - computes Q^T, K^T (layout [128 chan, T], head pair stacked 64+64 on
    partitions) and V ([T, 128+2 ones-cols]) for its head pair, all batches
  - scores S^T[tk, tq] = K_tile @ Q^T per (q-group of 512, key tile), both
    heads concurrently via PE row-tiling (contraction=64 each) into a
    2-bank PSUM tensor
  - P^T = exp(S^T/8) on ACT, one instruction per key tile covering both
    heads (no max subtraction needed: |scores| < ~8); causal mask applied
    multiplicatively on the diagonal 128x128 block
  - o^T[65, tq] += V_aug^T @ P^T accumulated in PSUM over key tiles; the
    appended ones-column of V_aug yields softmax denominators in row 64
  - normalize by transposing o^T to [tq, dh+1] (PE transpose), reciprocal
    of the per-partition denominator column, per-partition scalar multiply
  - out-proj partial = y^T.T @ Wp[slice] -> fp32 partial output

The PE instruction stream is software-pipelined: batch b's attention
steps are interleaved with batch b+1's Q/K/V projection chains and batch
b's out-projection chunks ("filler" units), keeping TensorE dense so the
HAM clock gate stays at full rate.

Host sums the 8 partial outputs and adds bp (the "all-reduce after proj"
done at unshard time). No device collectives.
"""

from collections import deque
from contextlib import ExitStack

import numpy as np
import ml_dtypes

import concourse.bass as bass
import concourse.tile as tile
from concourse import mybir
from concourse.bass import ts, ds
from concourse.bass_utils import run_bass_kernel_spmd
from concourse.masks import make_upper_triangular, make_identity

B, T, C, H = 4, 2048, 1024, 16
HD = C // H          # 64
NCORES = 8
CPC = C // NCORES    # 128 channels per core (2 heads)
P = 128
KT = C // P          # 8 contraction tiles for projections
NT = T // P          # 16 token tiles per batch
NG = T // 512        # 4 query groups of 512 tokens per batch
F32 = mybir.dt.float32
BF16 = mybir.dt.bfloat16

_CACHED = {}
LAST_RESULT = None


def _build_module(use_qk_bias: bool, use_v_bias: bool):
    from concourse import bacc
    nc = bacc.Bacc("TRN2", target_bir_lowering=False, debug=False,
                   num_devices=NCORES)

    xT = nc.dram_tensor("xT", (B, C, T), BF16, kind="ExternalInput").ap()
    wq = nc.dram_tensor("wq", (C, CPC), BF16, kind="ExternalInput").ap()
    wk = nc.dram_tensor("wk", (C, CPC), BF16, kind="ExternalInput").ap()
    wv = nc.dram_tensor("wv", (C, CPC), BF16, kind="ExternalInput").ap()
    wp = nc.dram_tensor("wp", (CPC, C), BF16, kind="ExternalInput").ap()
    bq = nc.dram_tensor("bq", (CPC, 1), F32, kind="ExternalInput").ap()
    bk = nc.dram_tensor("bk", (CPC, 1), F32, kind="ExternalInput").ap()
    bv = nc.dram_tensor("bv", (1, CPC), F32, kind="ExternalInput").ap()
    out = nc.dram_tensor("out", (B, T, C), F32, kind="ExternalOutput").ap()

    with tile.TileContext(nc) as tc:
        _emit(tc, xT, wq, wk, wv, wp, bq, bk, bv, out,
              use_qk_bias, use_v_bias)
    nc.compile()
    return nc


def _emit(tc, xT, wq, wk, wv, wp, bq, bk, bv, out, use_qk_bias, use_v_bias):
    nc = tc.nc
    Exp = mybir.ActivationFunctionType.Exp
    Copy = mybir.ActivationFunctionType.Copy
    Ident = mybir.ActivationFunctionType.Identity
    scale = 1.0 / np.sqrt(HD)

    ctx = ExitStack()
    consts = ctx.enter_context(tc.tile_pool(name="consts", bufs=1))
    pool_x = ctx.enter_context(tc.tile_pool(name="x", bufs=2))
    pool_kt = ctx.enter_context(tc.tile_pool(name="kt", bufs=2))
    pool_qt = ctx.enter_context(tc.tile_pool(name="qt", bufs=2))
    pool_v = ctx.enter_context(tc.tile_pool(name="v", bufs=2))
    pool_y = ctx.enter_context(tc.tile_pool(name="y", bufs=3))
    pool_p = ctx.enter_context(tc.tile_pool(name="p", bufs=4))
    pool_o = ctx.enter_context(tc.tile_pool(name="osb", bufs=3))
    pool_sm = ctx.enter_context(tc.tile_pool(name="small", bufs=4))
    ps_big = ctx.enter_context(tc.tile_pool(name="ps_big", bufs=2, space="PSUM"))
    ps_s = ctx.enter_context(tc.tile_pool(name="ps_s", bufs=2, space="PSUM"))
    ps_o = ctx.enter_context(tc.tile_pool(name="ps_o", bufs=1, space="PSUM"))

    # ---- constants ----
    # Multiplicative causal mask in [tk, tq] coords: 1 where tq >= tk.
    mask_ut = consts.tile([P, P], BF16)
    make_upper_triangular(nc, mask_ut[:], val=1.0, diag=True)

    identity = consts.tile([P, P], F32)
    make_identity(nc, identity[:])

    wq_sb = consts.tile([P, KT, CPC], BF16)
    nc.sync.dma_start(wq_sb[:], wq.rearrange("(kt p) c -> p kt c", p=P))
    wk_sb = consts.tile([P, KT, CPC], BF16)
    nc.sync.dma_start(wk_sb[:], wk.rearrange("(kt p) c -> p kt c", p=P))
    wv_sb = consts.tile([P, KT, CPC], BF16)
    nc.sync.dma_start(wv_sb[:], wv.rearrange("(kt p) c -> p kt c", p=P))
    wp_sb = consts.tile([P, C], BF16)
    nc.sync.dma_start(wp_sb[:], wp)

    if use_qk_bias:
        bq_sb = consts.tile([P, 1], F32)
        nc.sync.dma_start(bq_sb[:], bq)
        bk_sb = consts.tile([P, 1], F32)
        nc.sync.dma_start(bk_sb[:], bk)
    if use_v_bias:
        bv_row = consts.tile([1, CPC], F32)
        nc.sync.dma_start(bv_row[:], bv)
        ones_col = consts.tile([1, P], F32)
        nc.vector.memset(ones_col[:], 1.0)
        ps_bv = ps_big.tile([P, 512], F32, tag="big", name="ps_bv")
        nc.tensor.matmul(ps_bv[:, :CPC], ones_col[:], bv_row[:],
                         start=True, stop=True)
        bv_bc = consts.tile([P, CPC], F32)
        nc.vector.tensor_copy(bv_bc[:], ps_bv[:, :CPC])

    # per-batch resident tiles, allocated lazily (2 batches live at a time)
    state = {}

    def ensure_batch(b):
        if b in state:
            return state[b]
        st = {}
        st["x"] = pool_x.tile([P, KT, T], BF16, tag="x", name=f"x{b}")
        for k in range(KT):
            nc.sync.dma_start(
                st["x"][:, k],
                xT[b].rearrange("(kt p) t -> p kt t", p=P)[:, k])
        st["kt"] = pool_kt.tile([P, T], BF16, tag="kt", name=f"kt{b}")
        st["qt"] = pool_qt.tile([P, T], BF16, tag="qt", name=f"qt{b}")
        st["v"] = pool_v.tile([P, NT, 2 * (HD + 1)], BF16, tag="v",
                              name=f"v{b}")
        nc.vector.memset(st["v"][:, :, HD], 1.0)
        nc.vector.memset(st["v"][:, :, 2 * HD + 1], 1.0)
        st["y"] = pool_y.tile([P, NT, P], F32, tag="y", name=f"y{b}")
        state[b] = st
        return st

    def kq_unit(b, which, nchk):
        """One 512-column chunk of the K^T or Q^T projection of batch b."""
        def run():
            st = state[b]
            w_sb = wk_sb if which == "k" else wq_sb
            o_sb = st["kt"] if which == "k" else st["qt"]
            b_sb = None
            if use_qk_bias:
                b_sb = bk_sb if which == "k" else bq_sb
            ps = ps_big.tile([P, 512], F32, tag="big", name=f"ps_{which}")
            for k in range(KT):
                nc.tensor.matmul(ps[:], w_sb[:, k],
                                 st["x"][:, k, ts(nchk, 512)],
                                 start=(k == 0), stop=(k == KT - 1))
            if b_sb is not None:
                nc.scalar.activation(o_sb[:, ts(nchk, 512)], ps[:], Ident,
                                     bias=b_sb[:])
            elif which == "k":
                nc.scalar.activation(o_sb[:, ts(nchk, 512)], ps[:], Copy)
            else:
                nc.vector.tensor_copy(o_sb[:, ts(nchk, 512)], ps[:])
        return run

    def v_unit(b, tt):
        """One 128-token tile of the V projection of batch b."""
        def run():
            st = state[b]
            ps = ps_big.tile([P, 512], F32, tag="big", name="ps_v")
            for k in range(KT):
                nc.tensor.matmul(ps[:, :CPC], st["x"][:, k, ts(tt, P)],
                                 wv_sb[:, k],
                                 start=(k == 0), stop=(k == KT - 1))
            if use_v_bias:
                nc.vector.tensor_add(ps[:, :CPC], ps[:, :CPC], bv_bc[:])
            # one strided copy: psum [128,(2,64)] -> v cols {0:64, 65:129}
            dst = st["v"][:, tt, :].rearrange("p (h c) -> p h c", c=HD + 1)
            nc.vector.tensor_copy(
                dst[:, :, 0:HD],
                ps[:, :CPC].rearrange("p (h c) -> p h c", c=HD))
        return run

    def outproj_unit(b, tt):
        """Out-projection of one 128-token tile of batch b."""
        def run():
            st = state[b]
            yt_ps = ps_big.tile([P, P], F32, tag="big", name="yt_ps")
            nc.tensor.transpose(yt_ps[:], st["y"][:, tt, :], identity[:])
            ytsb = pool_p.tile([P, P], BF16, tag="p", name="ytsb")
            nc.vector.tensor_copy(ytsb[:], yt_ps[:])
            for nchk in range(C // 512):
                ps = ps_big.tile([P, 512], F32, tag="big", name="ps_op")
                nc.tensor.matmul(ps[:], ytsb[:], wp_sb[:, ts(nchk, 512)],
                                 start=True, stop=True)
                o_sb = pool_o.tile([P, 512], F32, tag="osb2", name="o_sb")
                if (tt + nchk) % 2 == 0:
                    nc.scalar.activation(o_sb[:], ps[:], Copy)
                else:
                    nc.vector.tensor_copy(o_sb[:], ps[:])
                nc.sync.dma_start(out[b, ts(tt, P), ts(nchk, 512)], o_sb[:])
        return run

    def proj_units(b):
        units = []
        for which in ("k", "q"):
            for nchk in range(T // 512):
                units.append(kq_unit(b, which, nchk))
        for tt in range(NT):
            units.append(v_unit(b, tt))
        return units

    def attn_batch(b, fillers_proj, fillers_free, allow_free):
        """Attention for batch b; pops one filler after each key-tile step
        and pushes out-projection units as query groups complete.
        Next-batch projection units (fillers_proj) are popped first; free
        units (out-projections) are popped only when allow_free, so they
        can be reserved as filler for the last batch."""
        st = state[b]
        kt_sb, qt_sb, v_sb, y_sb = st["kt"], st["qt"], st["v"], st["y"]

        def pop_filler():
            if fillers_proj:
                fillers_proj.popleft()()
            elif allow_free and fillers_free:
                fillers_free.popleft()()

        def emit_scores(j, g):
            qs = max(512 * g, P * j)
            nvis = 512 * (g + 1) - qs
            s_ps = ps_s.tile([P, 2, 512], F32, tag="s", name="s_ps")
            for h in (0, 1):
                hp = 64 * h
                nc.tensor.matmul(
                    s_ps[:, h, :nvis],
                    kt_sb[hp:hp + 64, ts(j, P)],
                    qt_sb[hp:hp + 64, ds(qs, nvis)],
                    start=True, stop=True,
                    tile_position=(hp, 0),
                )
            return s_ps

        for g in range(NG):
            o_ps = [ps_o.tile([HD + 1, 512], F32, tag=f"o{h}",
                              name=f"o_ps{h}") for h in (0, 1)]
            jmax = 4 * g + 3
            next_s = emit_scores(0, g)
            for j in range(jmax + 1):
                qs = max(512 * g, P * j)          # first visible query token
                nvis = 512 * (g + 1) - qs
                ql = qs - 512 * g
                diag = P * j >= 512 * g
                s_ps = next_s
                p_sb = pool_p.tile([P, 2, 512], BF16, tag="p", name="p_sb")
                nc.scalar.activation(p_sb[:, :, :nvis], s_ps[:, :, :nvis],
                                     Exp, scale=scale)
                if diag:
                    nc.vector.tensor_mul(
                        p_sb[:, :, 0:P], p_sb[:, :, 0:P],
                        mask_ut[:, None, :].to_broadcast([P, 2, P]))
                # PE filler + next scores run in exp_j's shadow, before AV_j
                pop_filler()
                if j < jmax:
                    next_s = emit_scores(j + 1, g)
                for h in (0, 1):
                    nc.tensor.matmul(
                        o_ps[h][:, ds(ql, nvis)],
                        v_sb[:, j, (HD + 1) * h:(HD + 1) * (h + 1)],
                        p_sb[:, h, :nvis],
                        start=(j == 0), stop=(j == jmax),
                    )
            # normalize: transpose o^T to [tq, dh+1]; row 64 holds the
            # softmax denominators -> per-partition scalars after transpose.
            for h in (0, 1):
                osb = pool_sm.tile([HD + 1, 512], F32, tag="osb", name="osb")
                nc.vector.tensor_copy(osb[:], o_ps[h][:])
                pop_filler()
                for q in range(4):
                    tt = 4 * g + q
                    ot_ps = ps_s.tile([P, HD + 1], F32, tag="s",
                                      name="ot_ps")
                    nc.tensor.transpose(ot_ps[:], osb[:, ts(q, P)],
                                        identity[0:HD + 1, 0:HD + 1])
                    rec = pool_sm.tile([P, 1], F32, tag="rec", name="rec")
                    nc.vector.reciprocal(rec[:], ot_ps[:, HD:HD + 1])
                    nc.vector.tensor_scalar_mul(
                        y_sb[:, tt, 64 * h:64 * h + HD],
                        ot_ps[:, 0:HD], rec[:])
            for tt in range(4 * g, 4 * g + 4):
                fillers_free.append(outproj_unit(b, tt))

    # ---- schedule ----
    ensure_batch(0)
    for u in proj_units(0):          # prologue: dense projections, warm PE
        u()
    fillers_free = deque()
    for b in range(B):
        fillers_proj = deque()
        if b + 1 < B:
            ensure_batch(b + 1)
            fillers_proj.extend(proj_units(b + 1))
        # reserve out-projection units during the second-to-last batch so
        # the last batch's attention still has dense PE filler
        attn_batch(b, fillers_proj, fillers_free, allow_free=(b != B - 2))
        while fillers_proj:
            fillers_proj.popleft()()
    while fillers_free:
        fillers_free.popleft()()

    ctx.close()


def _get_module(use_qk_bias, use_v_bias):
    key = (use_qk_bias, use_v_bias)
    if key not in _CACHED:
        _CACHED[key] = _build_module(*key)
    return _CACHED[key]


def kernel(x, Wq, bq, Wk, bk, Wv, bv, Wp, bp):
    bf16 = ml_dtypes.bfloat16
    x = np.asarray(x, np.float32)
    Wq = np.asarray(Wq, np.float32)
    Wk = np.asarray(Wk, np.float32)
    Wv = np.asarray(Wv, np.float32)
    Wp = np.asarray(Wp, np.float32)
    bq = np.asarray(bq, np.float32)
    bk = np.asarray(bk, np.float32)
    bv = np.asarray(bv, np.float32)
    bp = np.asarray(bp, np.float32)

    use_qk_bias = bool(np.any(bq) or np.any(bk))
    use_v_bias = bool(np.any(bv))
    nc = _get_module(use_qk_bias, use_v_bias)

    xT = np.ascontiguousarray(x.transpose(0, 2, 1)).astype(bf16)
    in_maps = []
    for c in range(NCORES):
        sl = slice(CPC * c, CPC * (c + 1))
        in_maps.append({
            "xT": xT,
            "wq": np.ascontiguousarray(Wq[:, sl]).astype(bf16),
            "wk": np.ascontiguousarray(Wk[:, sl]).astype(bf16),
            "wv": np.ascontiguousarray(Wv[:, sl]).astype(bf16),
            "wp": np.ascontiguousarray(Wp[sl, :]).astype(bf16),
            "bq": np.ascontiguousarray(bq[sl]).reshape(CPC, 1),
            "bk": np.ascontiguousarray(bk[sl]).reshape(CPC, 1),
            "bv": np.ascontiguousarray(bv[sl]).reshape(1, CPC),
        })

    res = run_bass_kernel_spmd(nc, in_maps, core_ids=list(range(NCORES)))
    global LAST_RESULT
    LAST_RESULT = res
    y = res.results[0]["out"].astype(np.float64)
    for c in range(1, NCORES):
        y += res.results[c]["out"]
    y += bp
    return y.astype(np.float32)
